# revision 11
# baseline (speedup 1.0000x reference)
"""ConcatSquashLinear + channel self-attention kernel for Trainium2 (8 NeuronCores).

Reference computation (per batch b; B=32, N=2048, Din=Dout=512, Dctx=256):
    gate = sigmoid(ctx @ W_gate.T + b_gate)            [1, Dout]
    bias = ctx @ W_bias.T                              [1, Dout]
    k    = ctx @ W_k.T                                 [1, Din]
    E    = outer(k, k)                                 [Din, Din] (symmetric)
    A    = softmax(E, axis=-1)                         row softmax
    A2   = A / (1e-9 + A.sum(axis=0))                  column renorm
    out  = ((x + x @ A2) @ W_layer.T) * gate + b_layer * gate + bias

Algebraic restructuring used here (all per batch):
    r_row[i] = 1 / sum_j exp(E[i,j])
    colsum[j] = sum_i exp(E[i,j]) * r_row[i]
    r_col[j] = 1 / (1e-9 + colsum[j])
    Wg[j,o]  = W_layer.T[j,o] * gate[o]
    Wg2      = diag(r_col) @ Wg
    Mtot     = Wg + diag(r_row) @ (expE @ Wg2)         [Din, Dout]
    c[o]     = b_layer[o] * gate[o] + bias[o]
    out      = x @ Mtot + c                            single big matmul per batch

Sharding: data-parallel over batch, 4 batches per core, weights replicated.
expE is symmetric, so its natural [i, j] tiles serve as the transposed
stationary operand for expE @ Wg2 without any physical transpose. Only x
needs on-chip transposition (PE transpose via identity).

float32r (reduced-precision fp32 matmul, 4x faster on the PE at N>=512) is
used for all matmul operands. Walrus requires every value consumed by an
f32r matmul to be *produced* with f32r rounding, so the operand tiles (and
the DRAM tensors DMA'd straight into them) are declared float32r end-to-end.
"""

import sys

import numpy as np

try:
    import concourse.bass as bass  # noqa: F401
except ImportError:  # pragma: no cover - path fallback for fresh dirs
    for _p in ("/opt/trn_rl_repo", "/root/.axon_site/_ro/trn_rl_repo"):
        if _p not in sys.path:
            sys.path.append(_p)
    import concourse.bass as bass  # noqa: F401

import concourse.tile as tile
from concourse import bacc, mybir
from concourse.bass_utils import run_bass_kernel_spmd
from concourse.masks import make_identity

B, N, DIN, DOUT, DCTX = 32, 2048, 512, 512, 256
NCORES = 8
BPC = B // NCORES      # batches per core
NT = N // 128          # 16 row-chunks of 128 points per batch
IC = DIN // 128        # 4 channel chunks
CC = DCTX // 128       # 2 ctx chunks

F32 = mybir.dt.float32
F32R = mybir.dt.float32r
AF = mybir.ActivationFunctionType


def build_program(mm_cast=True, tr_cast=True, copy_split=True):
    """Builds the per-core SPMD Bass program.

    mm_cast: use float32r for matmul operands (1 vs 4 cycles/row at N=512)
    tr_cast: use float32r for the PE transposes of x (1.5 vs 2 cycles/row)
    copy_split: route a share of PSUM->SBUF copies to ScalarE to unload DVE
    """
    DT = F32R if mm_cast else F32    # matmul operand dtype
    DTX = F32R if (mm_cast and tr_cast) else F32  # x / transpose dtype

    nc = bacc.Bacc("TRN2", target_bir_lowering=False, debug=False)

    x_d = nc.dram_tensor("x", [BPC, N, DIN], DTX, kind="ExternalInput")
    ctxT_d = nc.dram_tensor("ctxT", [DCTX, BPC], DT, kind="ExternalInput")
    wkT_d = nc.dram_tensor("wkT", [DCTX, DIN], DT, kind="ExternalInput")
    wgT_d = nc.dram_tensor("wgT", [DCTX, DOUT], DT, kind="ExternalInput")
    wbT_d = nc.dram_tensor("wbT", [DCTX, DOUT], DT, kind="ExternalInput")
    wlT_d = nc.dram_tensor("wlT", [DIN, DOUT], F32, kind="ExternalInput")
    bg_d = nc.dram_tensor("bg", [1, DOUT], DT, kind="ExternalInput")
    bl_d = nc.dram_tensor("bl", [1, DOUT], DT, kind="ExternalInput")
    out_d = nc.dram_tensor("out", [BPC, N, DOUT], F32, kind="ExternalOutput")

    with tile.TileContext(nc) as tc:
        with (
            tc.tile_pool(name="const", bufs=1) as const,
            tc.tile_pool(name="wpool", bufs=1) as wpool,
            tc.tile_pool(name="mpool", bufs=2) as mpool,
            tc.tile_pool(name="spool", bufs=2) as spool,
            tc.tile_pool(name="xpool", bufs=3) as xpool,
            tc.tile_pool(name="xtpool", bufs=3) as xtpool,
            tc.tile_pool(name="opool", bufs=3) as opool,
            tc.tile_pool(name="psum", bufs=1, space="PSUM") as psum,
        ):
            ones0 = const.tile([1, 128], F32)
            nc.vector.memset(ones0, 1.0)
            ones_sb = const.tile([1, 128], DT)
            nc.vector.tensor_copy(ones_sb, ones0)
            ident0 = const.tile([128, 128], F32)
            make_identity(nc, ident0)
            ident = const.tile([128, 128], DTX)
            nc.vector.tensor_copy(ident, ident0)

            wk_sb = wpool.tile([128, CC, DIN], DT)
            nc.sync.dma_start(out=wk_sb, in_=wkT_d.rearrange("(c p) i -> p c i", p=128))
            wg_sb = wpool.tile([128, CC, DOUT], DT)
            nc.sync.dma_start(out=wg_sb, in_=wgT_d.rearrange("(c p) i -> p c i", p=128))
            wb_sb = wpool.tile([128, CC, DOUT], DT)
            nc.sync.dma_start(out=wb_sb, in_=wbT_d.rearrange("(c p) i -> p c i", p=128))
            wl_sb = wpool.tile([128, IC, DOUT], F32)
            nc.sync.dma_start(out=wl_sb, in_=wlT_d.rearrange("(c p) o -> p c o", p=128))
            ctx_sb = wpool.tile([128, CC, BPC], DT)
            nc.sync.dma_start(out=ctx_sb, in_=ctxT_d.rearrange("(c p) b -> p c b", p=128))
            bg_sb = wpool.tile([1, DOUT], DT)
            nc.sync.dma_start(out=bg_sb, in_=bg_d[:, :])
            bl_sb = wpool.tile([1, DOUT], DT)
            nc.sync.dma_start(out=bl_sb, in_=bl_d[:, :])

            # ---- hyper-network projections (per batch, all on partition 0) ----
            k_sb = wpool.tile([1, BPC, DIN], DT)
            gate_sb = wpool.tile([1, BPC, DOUT], DT)
            c_sb = wpool.tile([1, BPC, DOUT], DT)
            ctmp_sb = wpool.tile([1, BPC, DOUT], F32)
            for b in range(BPC):
                kraw_ps = psum.tile([1, DIN], F32, tag="small", bufs=1)
                for c in range(CC):
                    nc.tensor.matmul(kraw_ps, ctx_sb[:, c, b:b + 1],
                                     wk_sb[:, c, :],
                                     start=(c == 0), stop=(c == CC - 1))
                nc.vector.tensor_copy(k_sb[:, b, :], kraw_ps)

                gpre_ps = psum.tile([1, DOUT], F32, tag="small", bufs=1)
                for c in range(CC):
                    nc.tensor.matmul(gpre_ps, ctx_sb[:, c, b:b + 1],
                                     wg_sb[:, c, :],
                                     start=(c == 0), stop=False)
                nc.tensor.matmul(gpre_ps, ones_sb[:, :1], bg_sb,
                                 start=False, stop=True)
                nc.scalar.activation(gate_sb[:, b, :], gpre_ps, AF.Sigmoid)

                bias_ps = psum.tile([1, DOUT], F32, tag="small", bufs=1)
                for c in range(CC):
                    nc.tensor.matmul(bias_ps, ctx_sb[:, c, b:b + 1],
                                     wb_sb[:, c, :],
                                     start=(c == 0), stop=(c == CC - 1))
                nc.vector.tensor_mul(ctmp_sb[:, b, :], gate_sb[:, b, :], bl_sb)
                nc.vector.tensor_add(c_sb[:, b, :], ctmp_sb[:, b, :], bias_ps)

            for b in range(BPC):
                # ---- attention precompute ----
                expE = [mpool.tile([128, DIN], DT, name=f"expE{d}", tag=f"expE{d}") for d in range(IC)]
                rs = spool.tile([128, IC], F32, tag="rs")
                for d in range(IC):
                    eng_ps = psum.tile([128, DIN], F32, tag="eng", bufs=2)
                    nc.tensor.matmul(eng_ps,
                                     k_sb[:, b, 128 * d:128 * (d + 1)],
                                     k_sb[:, b, :],
                                     start=True, stop=True)
                    nc.scalar.activation(expE[d], eng_ps, AF.Exp,
                                         accum_out=rs[:, d:d + 1])
                rrow_f = spool.tile([128, IC], F32, tag="rrow_f")
                nc.vector.reciprocal(rrow_f, rs)
                # f32r matmuls need even column counts -> keep r_row duplicated
                rrow = spool.tile([128, IC, 2], DT, tag="rrow")
                nc.vector.tensor_copy(rrow[:, :, 0], rrow_f)
                nc.vector.tensor_copy(rrow[:, :, 1], rrow_f)

                # column sums of attention (as column vectors per j-block)
                cs_ps = psum.tile([128, IC, 2], F32, tag="small", bufs=1)
                for d in range(IC):
                    for c in range(IC):
                        nc.tensor.matmul(cs_ps[:, d, :],
                                         expE[c][:, 128 * d:128 * (d + 1)],
                                         rrow[:, c, :],
                                         start=(c == 0), stop=(c == IC - 1))
                rcol = spool.tile([128, IC], F32, tag="rcol")
                cst = spool.tile([128, IC], F32, tag="cst")
                nc.vector.tensor_scalar_add(cst, cs_ps[:, :, 0], 1e-9)
                nc.vector.reciprocal(rcol, cst)

                # gate broadcast over 128 partitions; Wg, Wg2
                gb_ps = psum.tile([128, DOUT], F32, tag="small", bufs=1)
                nc.tensor.matmul(gb_ps, ones_sb, gate_sb[:, b, :],
                                 start=True, stop=True)
                wgt = [mpool.tile([128, DOUT], F32, name=f"wgt{d}", tag=f"wgt{d}") for d in range(IC)]
                wg2 = [mpool.tile([128, DOUT], DT, name=f"wg2{d}", tag=f"wg2{d}") for d in range(IC)]
                for d in range(IC):
                    nc.vector.tensor_mul(wgt[d], wl_sb[:, d, :], gb_ps)
                    nc.vector.tensor_scalar_mul(wg2[d], wgt[d], rcol[:, d:d + 1])

                # P = expE @ Wg2 (uses symmetry of expE); Mtot = Wg + r_row * P
                mtot = [mpool.tile([128, DOUT], DT, name=f"mtot{d}", tag=f"mtot{d}") for d in range(IC)]
                for d in range(IC):
                    p_ps = psum.tile([128, DOUT], F32, tag="p", bufs=1)
                    for c in range(IC):
                        nc.tensor.matmul(p_ps,
                                         expE[c][:, 128 * d:128 * (d + 1)],
                                         wg2[c],
                                         start=(c == 0), stop=(c == IC - 1))
                    ptmp = spool.tile([128, DOUT], F32, tag="ptmp")
                    nc.scalar.activation(ptmp, p_ps, AF.Copy, scale=rrow_f[:, d:d + 1])
                    nc.vector.tensor_add(mtot[d], ptmp, wgt[d])

                # ---- main pipeline over 16 row-chunks ----
                for t in range(NT):
                    xin = xpool.tile([128, DIN], DTX, tag="xin")
                    nc.sync.dma_start(out=xin, in_=x_d[b, 128 * t:128 * (t + 1), :])
                    xt_ps = psum.tile([128, DIN], DTX, tag="xt", bufs=2)
                    for c in range(IC):
                        nc.tensor.matmul(xt_ps[:, 128 * c:128 * (c + 1)],
                                         xin[:, 128 * c:128 * (c + 1)],
                                         ident, is_transpose=True)
                    xt_sb = xtpool.tile([128, DIN], DT, tag="xts")
                    nc.vector.tensor_copy(xt_sb, xt_ps)

                    o_ps = psum.tile([128, DOUT], F32, tag="ops", bufs=2)
                    for c in range(IC):
                        nc.tensor.matmul(o_ps, xt_sb[:, 128 * c:128 * (c + 1)],
                                         mtot[c], start=(c == 0), stop=False)
                    nc.tensor.matmul(o_ps, ones_sb, c_sb[:, b, :],
                                     start=False, stop=True)
                    o_sb = opool.tile([128, DOUT], F32, tag="osb")
                    if copy_split and t % 2 == 1:
                        nc.scalar.activation(o_sb, o_ps, AF.Copy)
                    else:
                        nc.vector.tensor_copy(o_sb, o_ps)
                    nc.sync.dma_start(out=out_d[b, 128 * t:128 * (t + 1), :], in_=o_sb)

    return nc


def prep_inputs(ctx, x, W_layer, b_layer, W_bias, W_gate, b_gate, W_k):
    """Host-side layout prep + per-core sharding. Returns in_maps for 8 cores."""
    f = np.float32
    wkT = np.ascontiguousarray(W_k.T, dtype=f)        # [DCTX, DIN]
    wgT = np.ascontiguousarray(W_gate.T, dtype=f)     # [DCTX, DOUT]
    wbT = np.ascontiguousarray(W_bias.T, dtype=f)     # [DCTX, DOUT]
    wlT = np.ascontiguousarray(W_layer.T, dtype=f)    # [DIN, DOUT]
    bg = np.ascontiguousarray(np.asarray(b_gate).reshape(1, DOUT), dtype=f)
    bl = np.ascontiguousarray(np.asarray(b_layer).reshape(1, DOUT), dtype=f)
    in_maps = []
    for core in range(NCORES):
        s = slice(core * BPC, (core + 1) * BPC)
        in_maps.append({
            "x": np.ascontiguousarray(x[s], dtype=f),
            "ctxT": np.ascontiguousarray(np.asarray(ctx)[s, 0, :].T, dtype=f),
            "wkT": wkT, "wgT": wgT, "wbT": wbT, "wlT": wlT,
            "bg": bg, "bl": bl,
        })
    return in_maps


def run(inputs, mm_cast=True, tr_cast=True, trace=False, **kw):
    nc = build_program(mm_cast=mm_cast, tr_cast=tr_cast)
    nc.finalize()
    in_maps = prep_inputs(**inputs)
    res = run_bass_kernel_spmd(nc, in_maps, list(range(NCORES)), trace=trace, **kw)
    out = np.concatenate([res.results[i]["out"] for i in range(NCORES)], axis=0)
    return out.astype(np.float32), res


def kernel(**inputs):
    out, _ = run(inputs)
    return out


# revision 16
# speedup vs baseline: 1.5791x; 1.5791x over previous
"""ConcatSquashLinear + channel self-attention kernel for Trainium2 (8 NeuronCores).

Reference computation (per batch b; B=32, N=2048, Din=Dout=512, Dctx=256):
    gate = sigmoid(ctx @ W_gate.T + b_gate)            [1, Dout]
    bias = ctx @ W_bias.T                              [1, Dout]
    k    = ctx @ W_k.T                                 [1, Din]
    E    = outer(k, k)                                 [Din, Din] (symmetric)
    A    = softmax(E, axis=-1)                         row softmax
    A2   = A / (1e-9 + A.sum(axis=0))                  column renorm
    out  = ((x + x @ A2) @ W_layer.T) * gate + b_layer * gate + bias

Algebraic restructuring used here (all per batch):
    r_row[i] = 1 / sum_j exp(E[i,j])
    colsum[j] = sum_i exp(E[i,j]) * r_row[i]
    r_col[j] = 1 / (1e-9 + colsum[j])
    Wg[j,o]  = W_layer.T[j,o] * gate[o]
    Wg2      = diag(r_col) @ Wg
    Mtot     = Wg + diag(r_row) @ (expE @ Wg2)         [Din, Dout]
    c[o]     = b_layer[o] * gate[o] + bias[o]
    out      = x @ Mtot + c                            single big matmul per batch

Sharding: data-parallel over batch, 4 batches per core, weights replicated.
expE is symmetric, so its natural [i, j] tiles serve as the transposed
stationary operand for expE @ Wg2 without any physical transpose. Only x
needs transposition (channel dim must land on partitions for the PE).

Two precision modes:
  "bf16": x / attention weights / Mtot in bfloat16. x is cast fp32->bf16
          in-flight by SWDGE DMA, transposed by the DMA xbar (2-byte path),
          and the big matmuls run at the PE's native 1 cycle/row with fast
          weight loads. The hyper-network and the softmax input (k, energy)
          stay in f32r/fp32 so only attention-weight-class values are bf16.
  "f32r": everything in float32r (reduced fp32, ~2 cycles/row measured,
          explicit fp32 LDWEIGHTS). ~2.4x slower, ~1.8e-4 max rel err.
"""

import sys

import numpy as np

try:
    import concourse.bass as bass  # noqa: F401
except ImportError:  # pragma: no cover - path fallback for fresh dirs
    for _p in ("/opt/trn_rl_repo", "/root/.axon_site/_ro/trn_rl_repo"):
        if _p not in sys.path:
            sys.path.append(_p)
    import concourse.bass as bass  # noqa: F401

import concourse.tile as tile
from concourse import bacc, mybir
from concourse.bass_utils import run_bass_kernel_spmd
from concourse.masks import make_identity

B, N, DIN, DOUT, DCTX = 32, 2048, 512, 512, 256
NCORES = 8
BPC = B // NCORES      # batches per core
NT = N // 128          # 16 row-chunks of 128 points per batch
IC = DIN // 128        # 4 channel chunks
CC = DCTX // 128       # 2 ctx chunks

F32 = mybir.dt.float32
F32R = mybir.dt.float32r
BF16 = mybir.dt.bfloat16
AF = mybir.ActivationFunctionType


def build_program(mode="bf16", copy_split=True):
    bf = mode == "bf16"
    DTM = BF16 if bf else F32R   # main-matmul operand dtype (x, Mtot, c)
    DTA = BF16 if bf else F32R   # attention-weight dtype (expE, rrow, wg2)

    nc = bacc.Bacc("TRN2", target_bir_lowering=False, debug=False)

    x_d = nc.dram_tensor("x", [BPC, N, DIN], F32 if bf else F32R,
                         kind="ExternalInput")
    ctxT_d = nc.dram_tensor("ctxT", [DCTX, BPC], F32R, kind="ExternalInput")
    wkT_d = nc.dram_tensor("wkT", [DCTX, DIN], F32R, kind="ExternalInput")
    wgT_d = nc.dram_tensor("wgT", [DCTX, DOUT], F32R, kind="ExternalInput")
    wbT_d = nc.dram_tensor("wbT", [DCTX, DOUT], F32R, kind="ExternalInput")
    wlT_d = nc.dram_tensor("wlT", [DIN, DOUT], F32, kind="ExternalInput")
    bg_d = nc.dram_tensor("bg", [1, DOUT], F32R, kind="ExternalInput")
    bl_d = nc.dram_tensor("bl", [1, DOUT], F32R, kind="ExternalInput")
    out_d = nc.dram_tensor("out", [BPC, N, DOUT], F32, kind="ExternalOutput")

    with tile.TileContext(nc) as tc:
        with (
            tc.tile_pool(name="const", bufs=1) as const,
            tc.tile_pool(name="wpool", bufs=1) as wpool,
            tc.tile_pool(name="mpool", bufs=2) as mpool,
            tc.tile_pool(name="spool", bufs=2) as spool,
            tc.tile_pool(name="xpool", bufs=3) as xpool,
            tc.tile_pool(name="xtpool", bufs=3) as xtpool,
            tc.tile_pool(name="opool", bufs=3) as opool,
            tc.tile_pool(name="psum", bufs=1, space="PSUM") as psum,
        ):
            ones0 = const.tile([1, 128], F32)
            nc.vector.memset(ones0, 1.0)
            ones_r = const.tile([1, 128], F32R)     # lhsT for f32r matmuls
            nc.vector.tensor_copy(ones_r, ones0)
            ones_m = const.tile([1, 128], DTM)      # lhsT for the +c matmul
            nc.vector.tensor_copy(ones_m, ones0)
            ident0 = const.tile([128, 128], F32)
            make_identity(nc, ident0)
            ident = const.tile([128, 128], DTM)
            nc.vector.tensor_copy(ident, ident0)

            wk_sb = wpool.tile([128, CC, DIN], F32R)
            nc.sync.dma_start(out=wk_sb, in_=wkT_d.rearrange("(c p) i -> p c i", p=128))
            wg_sb = wpool.tile([128, CC, DOUT], F32R)
            nc.sync.dma_start(out=wg_sb, in_=wgT_d.rearrange("(c p) i -> p c i", p=128))
            wb_sb = wpool.tile([128, CC, DOUT], F32R)
            nc.sync.dma_start(out=wb_sb, in_=wbT_d.rearrange("(c p) i -> p c i", p=128))
            wl_sb = wpool.tile([128, IC, DOUT], F32)
            nc.sync.dma_start(out=wl_sb, in_=wlT_d.rearrange("(c p) o -> p c o", p=128))
            ctx_sb = wpool.tile([128, CC, BPC], F32R)
            nc.sync.dma_start(out=ctx_sb, in_=ctxT_d.rearrange("(c p) b -> p c b", p=128))
            bg_sb = wpool.tile([1, DOUT], F32R)
            nc.sync.dma_start(out=bg_sb, in_=bg_d[:, :])
            bl_sb = wpool.tile([1, DOUT], F32R)
            nc.sync.dma_start(out=bl_sb, in_=bl_d[:, :])

            # ---- hyper-network projections (per batch, all on partition 0) ----
            k_sb = wpool.tile([1, BPC, DIN], F32R)
            gate_sb = wpool.tile([1, BPC, DOUT], F32R)
            c_sb = wpool.tile([1, BPC, DOUT], DTM)
            ctmp_sb = wpool.tile([1, BPC, DOUT], F32)
            for b in range(BPC):
                kraw_ps = psum.tile([1, DIN], F32, tag="small", bufs=1)
                for c in range(CC):
                    nc.tensor.matmul(kraw_ps, ctx_sb[:, c, b:b + 1],
                                     wk_sb[:, c, :],
                                     start=(c == 0), stop=(c == CC - 1))
                nc.vector.tensor_copy(k_sb[:, b, :], kraw_ps)

                gpre_ps = psum.tile([1, DOUT], F32, tag="small", bufs=1)
                for c in range(CC):
                    nc.tensor.matmul(gpre_ps, ctx_sb[:, c, b:b + 1],
                                     wg_sb[:, c, :],
                                     start=(c == 0), stop=False)
                nc.tensor.matmul(gpre_ps, ones_r[:, :1], bg_sb,
                                 start=False, stop=True)
                nc.scalar.activation(gate_sb[:, b, :], gpre_ps, AF.Sigmoid)

                bias_ps = psum.tile([1, DOUT], F32, tag="small", bufs=1)
                for c in range(CC):
                    nc.tensor.matmul(bias_ps, ctx_sb[:, c, b:b + 1],
                                     wb_sb[:, c, :],
                                     start=(c == 0), stop=(c == CC - 1))
                nc.vector.tensor_mul(ctmp_sb[:, b, :], gate_sb[:, b, :], bl_sb)
                nc.vector.tensor_add(c_sb[:, b, :], ctmp_sb[:, b, :], bias_ps)

            for b in range(BPC):
                # ---- attention precompute ----
                expE = [mpool.tile([128, DIN], DTA, name=f"expE{d}", tag=f"expE{d}") for d in range(IC)]
                rs = spool.tile([128, IC], F32, tag="rs")
                for d in range(IC):
                    eng_ps = psum.tile([128, DIN], F32, tag="eng", bufs=2)
                    nc.tensor.matmul(eng_ps,
                                     k_sb[:, b, 128 * d:128 * (d + 1)],
                                     k_sb[:, b, :],
                                     start=True, stop=True)
                    nc.scalar.activation(expE[d], eng_ps, AF.Exp,
                                         accum_out=rs[:, d:d + 1])
                rrow_f = spool.tile([128, IC], F32, tag="rrow_f")
                nc.vector.reciprocal(rrow_f, rs)
                # f32r matmuls need even column counts -> keep r_row duplicated
                rrow = spool.tile([128, IC, 2], DTA, tag="rrow")
                nc.vector.tensor_copy(rrow[:, :, 0], rrow_f)
                nc.vector.tensor_copy(rrow[:, :, 1], rrow_f)

                # column sums of attention (as column vectors per j-block)
                cs_ps = psum.tile([128, IC, 2], F32, tag="small", bufs=1)
                for d in range(IC):
                    for c in range(IC):
                        nc.tensor.matmul(cs_ps[:, d, :],
                                         expE[c][:, 128 * d:128 * (d + 1)],
                                         rrow[:, c, :],
                                         start=(c == 0), stop=(c == IC - 1))
                rcol = spool.tile([128, IC], F32, tag="rcol")
                cst = spool.tile([128, IC], F32, tag="cst")
                nc.vector.tensor_scalar_add(cst, cs_ps[:, :, 0], 1e-9)
                nc.vector.reciprocal(rcol, cst)

                # gate broadcast over 128 partitions; Wg, Wg2
                gb_ps = psum.tile([128, DOUT], F32, tag="small", bufs=1)
                nc.tensor.matmul(gb_ps, ones_r, gate_sb[:, b, :],
                                 start=True, stop=True)
                wgt = [mpool.tile([128, DOUT], F32, name=f"wgt{d}", tag=f"wgt{d}") for d in range(IC)]
                wg2 = [mpool.tile([128, DOUT], DTA, name=f"wg2{d}", tag=f"wg2{d}") for d in range(IC)]
                for d in range(IC):
                    nc.vector.tensor_mul(wgt[d], wl_sb[:, d, :], gb_ps)
                    nc.vector.tensor_scalar_mul(wg2[d], wgt[d], rcol[:, d:d + 1])

                # P = expE @ Wg2 (uses symmetry of expE); Mtot = Wg + r_row * P
                mtot = [mpool.tile([128, DOUT], DTM, name=f"mtot{d}", tag=f"mtot{d}") for d in range(IC)]
                for d in range(IC):
                    p_ps = psum.tile([128, DOUT], F32, tag="p", bufs=1)
                    for c in range(IC):
                        nc.tensor.matmul(p_ps,
                                         expE[c][:, 128 * d:128 * (d + 1)],
                                         wg2[c],
                                         start=(c == 0), stop=(c == IC - 1))
                    ptmp = spool.tile([128, DOUT], F32, tag="ptmp")
                    nc.scalar.activation(ptmp, p_ps, AF.Copy, scale=rrow_f[:, d:d + 1])
                    nc.vector.tensor_add(mtot[d], ptmp, wgt[d])

                # ---- main pipeline over 16 row-chunks ----
                for t in range(NT):
                    xin = xpool.tile([128, DIN], DTM, tag="xin")
                    if bf:
                        # SWDGE casts fp32->bf16 in flight
                        nc.gpsimd.dma_start(out=xin,
                                            in_=x_d[b, 128 * t:128 * (t + 1), :])
                    else:
                        nc.sync.dma_start(out=xin,
                                          in_=x_d[b, 128 * t:128 * (t + 1), :])
                    xt_ps = psum.tile([128, DIN], DTM, tag="xt", bufs=2)
                    for c in range(IC):
                        nc.tensor.matmul(xt_ps[:, 128 * c:128 * (c + 1)],
                                         xin[:, 128 * c:128 * (c + 1)],
                                         ident, is_transpose=True)
                    xt_sb = xtpool.tile([128, IC, 128], DTM, tag="xts")
                    nc.vector.tensor_copy(xt_sb.rearrange("p c n -> p (c n)"), xt_ps)

                    o_ps = psum.tile([128, DOUT], F32, tag="ops", bufs=2)
                    for c in range(IC):
                        nc.tensor.matmul(o_ps, xt_sb[:, c, :],
                                         mtot[c], start=(c == 0), stop=False)
                    nc.tensor.matmul(o_ps, ones_m, c_sb[:, b, :],
                                     start=False, stop=True)
                    o_sb = opool.tile([128, DOUT], F32, tag="osb")
                    if copy_split and t % 2 == 1:
                        nc.scalar.activation(o_sb, o_ps, AF.Copy)
                    else:
                        nc.vector.tensor_copy(o_sb, o_ps)
                    nc.sync.dma_start(out=out_d[b, 128 * t:128 * (t + 1), :], in_=o_sb)

    return nc


def prep_inputs(ctx, x, W_layer, b_layer, W_bias, W_gate, b_gate, W_k):
    """Host-side layout prep + per-core sharding. Returns in_maps for 8 cores."""
    f = np.float32
    wkT = np.ascontiguousarray(np.asarray(W_k).T, dtype=f)        # [DCTX, DIN]
    wgT = np.ascontiguousarray(np.asarray(W_gate).T, dtype=f)     # [DCTX, DOUT]
    wbT = np.ascontiguousarray(np.asarray(W_bias).T, dtype=f)     # [DCTX, DOUT]
    wlT = np.ascontiguousarray(np.asarray(W_layer).T, dtype=f)    # [DIN, DOUT]
    bg = np.ascontiguousarray(np.asarray(b_gate).reshape(1, DOUT), dtype=f)
    bl = np.ascontiguousarray(np.asarray(b_layer).reshape(1, DOUT), dtype=f)
    x = np.asarray(x)
    ctx = np.asarray(ctx)
    in_maps = []
    for core in range(NCORES):
        s = slice(core * BPC, (core + 1) * BPC)
        in_maps.append({
            "x": np.ascontiguousarray(x[s], dtype=f),
            "ctxT": np.ascontiguousarray(ctx[s, 0, :].T, dtype=f),
            "wkT": wkT, "wgT": wgT, "wbT": wbT, "wlT": wlT,
            "bg": bg, "bl": bl,
        })
    return in_maps


def run(inputs, mode="bf16", trace=False, **kw):
    nc = build_program(mode=mode)
    nc.finalize()
    in_maps = prep_inputs(**inputs)
    res = run_bass_kernel_spmd(nc, in_maps, list(range(NCORES)), trace=trace, **kw)
    out = np.concatenate([res.results[i]["out"] for i in range(NCORES)], axis=0)
    return out.astype(np.float32), res


def kernel(**inputs):
    out, _ = run(inputs)
    return out


# revision 18
# speedup vs baseline: 1.7819x; 1.1284x over previous
"""ConcatSquashLinear + channel self-attention kernel for Trainium2 (8 NeuronCores).

Reference computation (per batch b; B=32, N=2048, Din=Dout=512, Dctx=256):
    gate = sigmoid(ctx @ W_gate.T + b_gate)            [1, Dout]
    bias = ctx @ W_bias.T                              [1, Dout]
    k    = ctx @ W_k.T                                 [1, Din]
    E    = outer(k, k)                                 [Din, Din] (symmetric)
    A    = softmax(E, axis=-1)                         row softmax
    A2   = A / (1e-9 + A.sum(axis=0))                  column renorm
    out  = ((x + x @ A2) @ W_layer.T) * gate + b_layer * gate + bias

Algebraic restructuring used here (all per batch):
    r_row[i] = 1 / sum_j exp(E[i,j])
    colsum[j] = sum_i exp(E[i,j]) * r_row[i]
    r_col[j] = 1 / (1e-9 + colsum[j])
    Wg[j,o]  = W_layer.T[j,o] * gate[o]
    Wg2      = diag(r_col) @ Wg
    Mtot     = Wg + diag(r_row) @ (expE @ Wg2)         [Din, Dout]
    c[o]     = b_layer[o] * gate[o] + bias[o]
    out      = x @ Mtot + c                            single big matmul per batch

Sharding: data-parallel over batch, 4 batches per core, weights replicated.
expE is symmetric, so its natural [i, j] tiles serve as the transposed
stationary operand for expE @ Wg2 without any physical transpose. Only x
needs transposition (channel dim must land on partitions for the PE).

Two precision modes:
  "bf16": x / attention weights / Mtot in bfloat16. x is cast fp32->bf16
          in-flight by SWDGE DMA, transposed on the PE (1 cycle/row), and
          the big matmuls run at the PE's native bf16 rate with fast
          weight loads. The hyper-network and the softmax input (k, energy)
          stay in f32r/fp32 so only attention-weight-class values are bf16.
          Measured: ~213 us/core, 2.2e-3 max scale-relative error.
  "f32r": everything in float32r (reduced fp32, ~2 cycles/row measured,
          explicit fp32 LDWEIGHTS). Measured: ~336 us, 1.8e-4 max error.
"""

import sys

import numpy as np

try:
    import concourse.bass as bass  # noqa: F401
except ImportError:  # pragma: no cover - path fallback for fresh dirs
    for _p in ("/opt/trn_rl_repo", "/root/.axon_site/_ro/trn_rl_repo"):
        if _p not in sys.path:
            sys.path.append(_p)
    import concourse.bass as bass  # noqa: F401

import concourse.tile as tile
from concourse import bacc, mybir
from concourse.bass_utils import run_bass_kernel_spmd
from concourse.masks import make_identity

B, N, DIN, DOUT, DCTX = 32, 2048, 512, 512, 256
NCORES = 8
BPC = B // NCORES      # batches per core
NT = N // 128          # 16 row-chunks of 128 points per batch
IC = DIN // 128        # 4 channel chunks
CC = DCTX // 128       # 2 ctx chunks

F32 = mybir.dt.float32
F32R = mybir.dt.float32r
BF16 = mybir.dt.bfloat16
AF = mybir.ActivationFunctionType


def build_program(mode="bf16", copy_split=True):
    bf = mode == "bf16"
    DTM = BF16 if bf else F32R   # main-matmul operand dtype (x, Mtot, c)
    DTA = BF16 if bf else F32R   # attention-weight dtype (expE, rrow, wg2)

    nc = bacc.Bacc("TRN2", target_bir_lowering=False, debug=False)

    x_d = nc.dram_tensor("x", [BPC, N, DIN], F32 if bf else F32R,
                         kind="ExternalInput")
    ctxT_d = nc.dram_tensor("ctxT", [DCTX, BPC], F32R, kind="ExternalInput")
    wkT_d = nc.dram_tensor("wkT", [DCTX, DIN], F32R, kind="ExternalInput")
    wgT_d = nc.dram_tensor("wgT", [DCTX, DOUT], F32R, kind="ExternalInput")
    wbT_d = nc.dram_tensor("wbT", [DCTX, DOUT], F32R, kind="ExternalInput")
    wlT_d = nc.dram_tensor("wlT", [DIN, DOUT], F32, kind="ExternalInput")
    bg_d = nc.dram_tensor("bg", [1, DOUT], F32R, kind="ExternalInput")
    bl_d = nc.dram_tensor("bl", [1, DOUT], F32R, kind="ExternalInput")
    out_d = nc.dram_tensor("out", [BPC, N, DOUT], F32, kind="ExternalOutput")

    with tile.TileContext(nc) as tc:
        with (
            tc.tile_pool(name="const", bufs=1) as const,
            tc.tile_pool(name="wpool", bufs=1) as wpool,
            tc.tile_pool(name="mpool", bufs=3) as mpool,
            tc.tile_pool(name="spool", bufs=3) as spool,
            tc.tile_pool(name="xpool", bufs=4) as xpool,
            tc.tile_pool(name="xtpool", bufs=4) as xtpool,
            tc.tile_pool(name="opool", bufs=4) as opool,
            tc.tile_pool(name="psum", bufs=1, space="PSUM") as psum,
        ):
            ones0 = const.tile([1, 128], F32)
            nc.vector.memset(ones0, 1.0)
            ones_r = const.tile([1, 128], F32R)     # lhsT for f32r matmuls
            nc.vector.tensor_copy(ones_r, ones0)
            ones_m = const.tile([1, 128], DTM)      # lhsT for the +c matmul
            nc.vector.tensor_copy(ones_m, ones0)
            ident0 = const.tile([128, 128], F32)
            make_identity(nc, ident0)
            ident = const.tile([128, 128], DTM)
            nc.vector.tensor_copy(ident, ident0)

            wk_sb = wpool.tile([128, CC, DIN], F32R)
            nc.sync.dma_start(out=wk_sb, in_=wkT_d.rearrange("(c p) i -> p c i", p=128))
            wg_sb = wpool.tile([128, CC, DOUT], F32R)
            nc.sync.dma_start(out=wg_sb, in_=wgT_d.rearrange("(c p) i -> p c i", p=128))
            wb_sb = wpool.tile([128, CC, DOUT], F32R)
            nc.sync.dma_start(out=wb_sb, in_=wbT_d.rearrange("(c p) i -> p c i", p=128))
            wl_sb = wpool.tile([128, IC, DOUT], F32)
            nc.sync.dma_start(out=wl_sb, in_=wlT_d.rearrange("(c p) o -> p c o", p=128))
            ctx_sb = wpool.tile([128, CC, BPC], F32R)
            nc.sync.dma_start(out=ctx_sb, in_=ctxT_d.rearrange("(c p) b -> p c b", p=128))
            bg_sb = wpool.tile([1, DOUT], F32R)
            nc.sync.dma_start(out=bg_sb, in_=bg_d[:, :])
            bl_sb = wpool.tile([1, DOUT], F32R)
            nc.sync.dma_start(out=bl_sb, in_=bl_d[:, :])

            # ---- hyper-network projections (per batch, all on partition 0) ----
            k_sb = wpool.tile([1, BPC, DIN], F32R)
            gate_sb = wpool.tile([1, BPC, DOUT], F32R)
            c_sb = wpool.tile([1, BPC, DOUT], DTM)
            ctmp_sb = wpool.tile([1, BPC, DOUT], F32)
            for b in range(BPC):
                kraw_ps = psum.tile([1, DIN], F32, tag="small", bufs=1)
                for c in range(CC):
                    nc.tensor.matmul(kraw_ps, ctx_sb[:, c, b:b + 1],
                                     wk_sb[:, c, :],
                                     start=(c == 0), stop=(c == CC - 1))
                nc.vector.tensor_copy(k_sb[:, b, :], kraw_ps)

                gpre_ps = psum.tile([1, DOUT], F32, tag="small", bufs=1)
                for c in range(CC):
                    nc.tensor.matmul(gpre_ps, ctx_sb[:, c, b:b + 1],
                                     wg_sb[:, c, :],
                                     start=(c == 0), stop=False)
                nc.tensor.matmul(gpre_ps, ones_r[:, :1], bg_sb,
                                 start=False, stop=True)
                nc.scalar.activation(gate_sb[:, b, :], gpre_ps, AF.Sigmoid)

                bias_ps = psum.tile([1, DOUT], F32, tag="small", bufs=1)
                for c in range(CC):
                    nc.tensor.matmul(bias_ps, ctx_sb[:, c, b:b + 1],
                                     wb_sb[:, c, :],
                                     start=(c == 0), stop=(c == CC - 1))
                nc.vector.tensor_mul(ctmp_sb[:, b, :], gate_sb[:, b, :], bl_sb)
                nc.vector.tensor_add(c_sb[:, b, :], ctmp_sb[:, b, :], bias_ps)

            for b in range(BPC):
                # ---- attention precompute ----
                expE = [mpool.tile([128, DIN], DTA, name=f"expE{d}", tag=f"expE{d}") for d in range(IC)]
                rs = spool.tile([128, IC], F32, tag="rs")
                for d in range(IC):
                    eng_ps = psum.tile([128, DIN], F32, tag="eng", bufs=1)
                    nc.tensor.matmul(eng_ps,
                                     k_sb[:, b, 128 * d:128 * (d + 1)],
                                     k_sb[:, b, :],
                                     start=True, stop=True)
                    nc.scalar.activation(expE[d], eng_ps, AF.Exp,
                                         accum_out=rs[:, d:d + 1])
                rrow_f = spool.tile([128, IC], F32, tag="rrow_f")
                nc.vector.reciprocal(rrow_f, rs)
                # f32r matmuls need even column counts -> keep r_row duplicated
                rrow = spool.tile([128, IC, 2], DTA, tag="rrow")
                nc.vector.tensor_copy(rrow[:, :, 0], rrow_f)
                nc.vector.tensor_copy(rrow[:, :, 1], rrow_f)

                # column sums of attention (as column vectors per j-block)
                cs_ps = psum.tile([128, IC, 2], F32, tag="small", bufs=1)
                for d in range(IC):
                    for c in range(IC):
                        nc.tensor.matmul(cs_ps[:, d, :],
                                         expE[c][:, 128 * d:128 * (d + 1)],
                                         rrow[:, c, :],
                                         start=(c == 0), stop=(c == IC - 1))
                rcol = spool.tile([128, IC], F32, tag="rcol")
                cst = spool.tile([128, IC], F32, tag="cst")
                nc.vector.tensor_scalar_add(cst, cs_ps[:, :, 0], 1e-9)
                nc.vector.reciprocal(rcol, cst)

                # gate broadcast over 128 partitions; Wg, Wg2
                gb_ps = psum.tile([128, DOUT], F32, tag="small", bufs=1)
                nc.tensor.matmul(gb_ps, ones_r, gate_sb[:, b, :],
                                 start=True, stop=True)
                wgt = [mpool.tile([128, DOUT], F32, name=f"wgt{d}", tag=f"wgt{d}") for d in range(IC)]
                wg2 = [mpool.tile([128, DOUT], DTA, name=f"wg2{d}", tag=f"wg2{d}") for d in range(IC)]
                for d in range(IC):
                    nc.vector.tensor_mul(wgt[d], wl_sb[:, d, :], gb_ps)
                    nc.vector.tensor_scalar_mul(wg2[d], wgt[d], rcol[:, d:d + 1])

                # P = expE @ Wg2 (uses symmetry of expE); Mtot = Wg + r_row * P
                mtot = [mpool.tile([128, DOUT], DTM, name=f"mtot{d}", tag=f"mtot{d}") for d in range(IC)]
                for d in range(IC):
                    p_ps = psum.tile([128, DOUT], F32, tag="p", bufs=2)
                    for c in range(IC):
                        nc.tensor.matmul(p_ps,
                                         expE[c][:, 128 * d:128 * (d + 1)],
                                         wg2[c],
                                         start=(c == 0), stop=(c == IC - 1))
                    ptmp = spool.tile([128, DOUT], F32, tag="ptmp")
                    nc.scalar.activation(ptmp, p_ps, AF.Copy, scale=rrow_f[:, d:d + 1])
                    nc.vector.tensor_add(mtot[d], ptmp, wgt[d])

                # ---- main pipeline over 16 row-chunks ----
                for t in range(NT):
                    xin = xpool.tile([128, DIN], DTM, tag="xin")
                    if bf:
                        # SWDGE casts fp32->bf16 in flight
                        nc.gpsimd.dma_start(out=xin,
                                            in_=x_d[b, 128 * t:128 * (t + 1), :])
                    else:
                        nc.sync.dma_start(out=xin,
                                          in_=x_d[b, 128 * t:128 * (t + 1), :])
                    xt_ps = psum.tile([128, DIN], DTM, tag="xt", bufs=2)
                    for c in range(IC):
                        nc.tensor.matmul(xt_ps[:, 128 * c:128 * (c + 1)],
                                         xin[:, 128 * c:128 * (c + 1)],
                                         ident, is_transpose=True)
                    xt_sb = xtpool.tile([128, IC, 128], DTM, tag="xts")
                    nc.vector.tensor_copy(xt_sb.rearrange("p c n -> p (c n)"), xt_ps)

                    o_ps = psum.tile([128, DOUT], F32, tag="ops", bufs=2)
                    for c in range(IC):
                        nc.tensor.matmul(o_ps, xt_sb[:, c, :],
                                         mtot[c], start=(c == 0), stop=False)
                    nc.tensor.matmul(o_ps, ones_m, c_sb[:, b, :],
                                     start=False, stop=True)
                    o_sb = opool.tile([128, DOUT], F32, tag="osb")
                    if copy_split and t % 2 == 1:
                        nc.scalar.activation(o_sb, o_ps, AF.Copy)
                    else:
                        nc.vector.tensor_copy(o_sb, o_ps)
                    nc.sync.dma_start(out=out_d[b, 128 * t:128 * (t + 1), :], in_=o_sb)

    return nc


def prep_inputs(ctx, x, W_layer, b_layer, W_bias, W_gate, b_gate, W_k):
    """Host-side layout prep + per-core sharding. Returns in_maps for 8 cores."""
    f = np.float32
    wkT = np.ascontiguousarray(np.asarray(W_k).T, dtype=f)        # [DCTX, DIN]
    wgT = np.ascontiguousarray(np.asarray(W_gate).T, dtype=f)     # [DCTX, DOUT]
    wbT = np.ascontiguousarray(np.asarray(W_bias).T, dtype=f)     # [DCTX, DOUT]
    wlT = np.ascontiguousarray(np.asarray(W_layer).T, dtype=f)    # [DIN, DOUT]
    bg = np.ascontiguousarray(np.asarray(b_gate).reshape(1, DOUT), dtype=f)
    bl = np.ascontiguousarray(np.asarray(b_layer).reshape(1, DOUT), dtype=f)
    x = np.asarray(x)
    ctx = np.asarray(ctx)
    in_maps = []
    for core in range(NCORES):
        s = slice(core * BPC, (core + 1) * BPC)
        in_maps.append({
            "x": np.ascontiguousarray(x[s], dtype=f),
            "ctxT": np.ascontiguousarray(ctx[s, 0, :].T, dtype=f),
            "wkT": wkT, "wgT": wgT, "wbT": wbT, "wlT": wlT,
            "bg": bg, "bl": bl,
        })
    return in_maps


def run(inputs, mode="bf16", trace=False, **kw):
    nc = build_program(mode=mode)
    nc.finalize()
    in_maps = prep_inputs(**inputs)
    res = run_bass_kernel_spmd(nc, in_maps, list(range(NCORES)), trace=trace, **kw)
    out = np.concatenate([res.results[i]["out"] for i in range(NCORES)], axis=0)
    return out.astype(np.float32), res


def kernel(**inputs):
    out, _ = run(inputs)
    return out
